# revision 28
# baseline (speedup 1.0000x reference)
"""Trainium2 Bass kernel for a beam tree-ensemble (256 complete binary trees,
depth 10, 256 features, 8 classes, batch 32768), data-parallel over batch on
8 NeuronCores.

Device algorithm (per core, 4096 samples):
  The traversal is made gather-free by computing, for every tree, the
  comparison bit x[b, feat[n]] >= thr[n] at ALL 1023 internal nodes via a
  one-hot matmul on the tensor engine (fp32 transpose-mode pass-through,
  bit-exact), then selecting the leaf with a level-by-level one-hot
  reachability recursion on the vector engine.

  Engine balance: PSUM compare-drains (is_ge) and the recursion run on
  VectorE; the level-9 fold uses a fused tensor_tensor_reduce (multiply +
  free-dim reduce in one op); table prep (u8->f32 cast, one-hot build) and
  leaf-index bit-packing run on the otherwise-idle GpSimd engine.

  Tables are host-permuted into level-major, bit-reversed-position order so
  the recursion writes contiguous child blocks. Feature/threshold tables are
  shipped as per-core shards (32 trees each) and AllGathered on device over
  NeuronLink. The device emits each (sample, tree) 10-bit leaf offset packed
  as a low byte plus four 2-bit highs per byte (1.25 B/leaf); the host
  performs the trivial per-row lookup values[tree, leaf] to materialize the
  [B, T, 8] output.
"""
import sys

sys.path.insert(0, "/opt/trn_rl_repo")

import numpy as np
import ml_dtypes

import concourse.bass as bass
import concourse.tile as tile
from concourse import bacc, mybir, bass_utils
import bass_rust

NUM_TREES = 256
MAX_TREE_DEPTH = 10
NUM_NODES = 2 ** (MAX_TREE_DEPTH + 1) - 1  # 2047
N_FEATURES = 256
N_CLASSES = 8
BATCH = 32768
N_CORES = 8
BC = BATCH // N_CORES  # 4096
NBT = 32  # 128-sample btiles covering the whole per-core batch
NCOL = 1024  # padded level-order columns

F32 = mybir.dt.float32
BF16 = mybir.dt.bfloat16
FP16 = mybir.dt.float16
I32 = mybir.dt.int32
U8 = mybir.dt.uint8
U16 = mybir.dt.uint16

_PROGRAM_CACHE = {}
SIM_MODE = False
SKIP_COLLECTIVES = False
USE_TTR = True
W_INPLACE = True
PS_BUFS = 8  # psum tiles are [128, 512] = 1 bank each


def _split_multi_waits(nc):
    """This walrus build accepts at most one sem-wait per instruction; move
    extra waits onto single-wait NoOps placed before the owner."""
    ctr = 0
    for bb in nc.m.functions[0].blocks:
        new = []
        changed = False
        for inst in bb.instructions:
            si = inst.sync_info
            if si is not None and si.on_wait and len(si.on_wait) > 1:
                waits = list(si.on_wait)
                for w in waits[:-1]:
                    ctr += 1
                    n = mybir.InstNoOp(name=f"WSPLIT-{ctr}", ins=[], outs=[])
                    n.engine = inst.engine
                    n.sync_info = bass_rust.SyncInfo(on_wait=[w], on_update=[])
                    new.append(n)
                si.on_wait = [waits[-1]]
                changed = True
            new.append(inst)
        if changed:
            bb.instructions = new


def _bcast_ap(ap, parts=128):
    return bass.AP(tensor=ap.tensor, offset=ap.offset, ap=[[0, parts]] + list(ap.ap))


# level-order column offsets: [levels 0-6 packed: 0..126][pad 127][l7: 128]
# [l8: 256][l9: 512]
_OFF = {d: (2 ** d - 1) for d in range(7)}
_OFF[7], _OFF[8], _OFF[9] = 128, 256, 512


def build_program():
    nc = bacc.Bacc("TRN2", debug=False)
    x_d = nc.dram_tensor("xt", [2, 128, BC], F32, kind="ExternalInput").ap()
    tpc = NUM_TREES // N_CORES  # table shard: trees per core
    feat_d = nc.dram_tensor("feat", [tpc, NCOL], U8, kind="ExternalInput").ap()
    thr_d = nc.dram_tensor("thr", [tpc, NCOL], F32, kind="ExternalInput").ap()
    featg_in = nc.dram_tensor("featg_in", [tpc, NCOL], U8).ap()
    thrg_in = nc.dram_tensor("thrg_in", [tpc, NCOL], F32).ap()
    feat_full = nc.dram_tensor("feat_full", [NUM_TREES, NCOL], U8).ap()
    thr_full = nc.dram_tensor("thr_full", [NUM_TREES, NCOL], F32).ap()
    pv_d = nc.dram_tensor("pv", [512], FP16, kind="ExternalInput").ap()
    olo_d = nc.dram_tensor("olo", [128, NBT, NUM_TREES], U8, kind="ExternalOutput").ap()
    ohi_d = nc.dram_tensor(
        "ohi", [128, NBT, NUM_TREES // 4], U8, kind="ExternalOutput"
    ).ap()

    with tile.TileContext(nc) as tc:
        with tile.ExitStack() as ctx:
            sb = ctx.enter_context(tc.tile_pool(name="sb", bufs=1))
            tpool = ctx.enter_context(tc.tile_pool(name="tp", bufs=2))
            opool = ctx.enter_context(tc.tile_pool(name="op", bufs=1))
            ps = ctx.enter_context(tc.tile_pool(name="ps", bufs=PS_BUFS, space="PSUM"))

            xsb = sb.tile([128, 2, BC], F32)
            nc.sync.dma_start(out=xsb, in_=x_d.rearrange("k p b -> p k b"))
            pvb = sb.tile([128, 512], FP16)
            nc.sync.dma_start(out=pvb, in_=_bcast_ap(pv_d))
            # gather the replicated tree tables from per-core shards
            if SIM_MODE or SKIP_COLLECTIVES:
                nc.sync.dma_start(out=feat_full, in_=feat_d)
                nc.sync.dma_start(out=thr_full, in_=thr_d)
            else:
                nc.sync.dma_start(out=featg_in, in_=feat_d)
                nc.sync.dma_start(out=thrg_in, in_=thr_d)
                nc.gpsimd.collective_compute(
                    "AllGather", mybir.AluOpType.bypass,
                    replica_groups=[list(range(N_CORES))],
                    ins=[featg_in.opt()], outs=[feat_full.opt()],
                )
                nc.gpsimd.collective_compute(
                    "AllGather", mybir.AluOpType.bypass,
                    replica_groups=[list(range(N_CORES))],
                    ins=[thrg_in.opt()], outs=[thr_full.opt()],
                )
            iota_i = sb.tile([128, 1], I32)
            iota_f = sb.tile([128, 2], F32)
            nc.gpsimd.iota(iota_i, pattern=[[0, 1]], base=0, channel_multiplier=1)
            nc.vector.tensor_copy(out=iota_f[:, 0:1], in_=iota_i)
            nc.vector.tensor_scalar_add(iota_f[:, 1:2], iota_f[:, 0:1], 128.0)
            ones = sb.tile([128, NBT // 2], FP16)
            nc.vector.memset(ones, 1.0)
            shc = sb.tile([128, 3], U16)
            for k, v in enumerate((2, 4, 6)):
                nc.vector.memset(shc[:, k : k + 1], v)
            ones_ap = bass.AP(
                tensor=ones[:].tensor, offset=ones[:].offset,
                ap=list(ones[:].ap) + [[0, 1]],
            )
            pv_ap = bass.AP(
                tensor=pvb[:].tensor, offset=pvb[:].offset,
                ap=[pvb[:].ap[0], [0, NBT // 2], pvb[:].ap[1]],
            )
            cpool = ctx.enter_context(tc.tile_pool(name="cp", bufs=2))
            oidx_sb = opool.tile([128, NBT, NUM_TREES], U16, tag="oidx")

            def tree_body(i):
                featb_u8 = tpool.tile([128, NCOL], U8, tag="featb_u8")
                featb = tpool.tile([128, NCOL], F32, tag="featb")
                thrb = tpool.tile([128, NCOL], F32, tag="thrb")
                g = tpool.tile([128, 2, NCOL], F32, tag="g")
                nc.sync.dma_start(
                    out=featb_u8, in_=_bcast_ap(feat_full[bass.ds(i, 1), :])
                )
                nc.scalar.copy(out=featb, in_=featb_u8)
                nc.sync.dma_start(
                    out=thrb, in_=_bcast_ap(thr_full[bass.ds(i, 1), :])
                )
                for kt in range(2):
                    nc.vector.tensor_scalar(
                        out=g[:, kt, :], in0=featb,
                        scalar1=iota_f[:, kt : kt + 1], scalar2=None,
                        op0=mybir.AluOpType.is_equal,
                    )
                NH = NBT // 2
                for half in range(2):
                    cmp = cpool.tile([128, NH, NCOL], FP16, tag="cmp")
                    rr = cpool.tile([128, NH, 512], FP16, tag="rr")
                    for bt in range(NH):
                        b0 = (half * NH + bt) * 128
                        for nch in range(2):
                            psum = ps.tile([128, 512], F32, tag="ps")
                            for kt in range(2):
                                nc.tensor.matmul(
                                    psum,
                                    xsb[:, kt, b0 : b0 + 128],
                                    g[:, kt, nch * 512 : (nch + 1) * 512],
                                    is_transpose=True,
                                    start=(kt == 0),
                                    stop=(kt == 1),
                                )
                            nc.vector.tensor_tensor(
                                out=cmp[:, bt, nch * 512 : (nch + 1) * 512],
                                in0=psum,
                                in1=thrb[:, nch * 512 : (nch + 1) * 512],
                                op=mybir.AluOpType.is_ge,
                            )
                    # reachability recursion: d=0 on VectorE (broadcast src),
                    # d>=1 on the otherwise-idle GpSimd engine
                    for d in range(9):
                        nd = 2 ** d
                        src = ones_ap if d == 0 else rr[:, :, 0:nd]
                        eng = nc.vector if d == 0 else nc.gpsimd
                        eng.tensor_tensor(
                            out=rr[:, :, nd : 2 * nd], in0=src,
                            in1=cmp[:, :, _OFF[d] : _OFF[d] + nd],
                            op=mybir.AluOpType.mult,
                        )
                        eng.tensor_tensor(
                            out=rr[:, :, 0:nd], in0=src,
                            in1=rr[:, :, nd : 2 * nd],
                            op=mybir.AluOpType.subtract,
                        )
                    # level-9 fold: w = C9 + pv (in place); u = w * R9;
                    # log2 halving-add reduce down to one column
                    nc.vector.tensor_tensor(
                        out=cmp[:, :, 512:1024], in0=cmp[:, :, 512:1024],
                        in1=pv_ap, op=mybir.AluOpType.add,
                    )
                    nc.vector.tensor_tensor(
                        out=cmp[:, :, 512:1024], in0=cmp[:, :, 512:1024],
                        in1=rr[:, :, 0:512], op=mybir.AluOpType.mult,
                    )
                    wd = 256
                    while wd >= 1:
                        nc.vector.tensor_tensor(
                            out=cmp[:, :, 512 : 512 + wd],
                            in0=cmp[:, :, 512 : 512 + wd],
                            in1=cmp[:, :, 512 + wd : 512 + 2 * wd],
                            op=mybir.AluOpType.add,
                        )
                        wd //= 2
                    nc.vector.tensor_copy(
                        out=oidx_sb[:, half * NH : (half + 1) * NH, bass.ds(i, 1)],
                        in_=cmp[:, :, 512:513],
                    )

            tc.For_i_unrolled(0, NUM_TREES, 1, tree_body, max_unroll=2)

            # pack 10-bit leaf offsets: low byte + 2-bit highs (4 trees/byte),
            # in 8-tree chunks on GpSimd
            olo = opool.tile([128, NBT, NUM_TREES], U8, tag="olo")
            ohp = opool.tile([128, NBT, NUM_TREES // 4], U8, tag="ohp")
            CH = 8  # trees per packing chunk
            hic = opool.tile([128, NBT, CH], U16, tag="hic")
            loc = opool.tile([128, NBT, CH], U16, tag="loc")
            pkc = opool.tile([128, NBT, CH // 4], U16, tag="pkc")
            for c in range(NUM_TREES // CH):
                t0 = c * CH
                nc.vector.tensor_scalar(
                    out=hic, in0=oidx_sb[:, :, t0 : t0 + CH], scalar1=8,
                    scalar2=None,
                    op0=mybir.AluOpType.logical_shift_right,
                )
                nc.vector.tensor_scalar(
                    out=loc, in0=oidx_sb[:, :, t0 : t0 + CH],
                    scalar1=255, scalar2=None,
                    op0=mybir.AluOpType.bitwise_and,
                )
                nc.vector.tensor_copy(out=olo[:, :, t0 : t0 + CH], in_=loc)
                nc.vector.scalar_tensor_tensor(
                    out=pkc, in0=hic[:, :, 1::4], scalar=shc[:, 0:1],
                    in1=hic[:, :, 0::4],
                    op0=mybir.AluOpType.logical_shift_left,
                    op1=mybir.AluOpType.bitwise_or,
                )
                nc.vector.scalar_tensor_tensor(
                    out=pkc, in0=hic[:, :, 2::4], scalar=shc[:, 1:2], in1=pkc,
                    op0=mybir.AluOpType.logical_shift_left,
                    op1=mybir.AluOpType.bitwise_or,
                )
                nc.vector.scalar_tensor_tensor(
                    out=pkc, in0=hic[:, :, 3::4], scalar=shc[:, 2:3], in1=pkc,
                    op0=mybir.AluOpType.logical_shift_left,
                    op1=mybir.AluOpType.bitwise_or,
                )
                nc.vector.tensor_copy(out=ohp[:, :, t0 // 4 : t0 // 4 + CH // 4], in_=pkc)
            nc.sync.dma_start(out=olo_d, in_=olo)
            nc.sync.dma_start(out=ohi_d, in_=ohp)
    nc.compile()
    if not SIM_MODE:
        _split_multi_waits(nc)
    return nc


def _bitrev_table(bits):
    n = 1 << bits
    t = np.zeros(n, np.int64)
    for p in range(n):
        r = 0
        for i in range(bits):
            r = (r << 1) | ((p >> i) & 1)
        t[p] = r
    return t


def prep_tables(features, thresholds):
    """Host-side: level-major bit-reversed column order + padding."""
    br = {d: _bitrev_table(d) for d in range(MAX_TREE_DEPTH)}
    colidx = np.concatenate(
        [2 ** d - 1 + br[d] for d in range(MAX_TREE_DEPTH)]
    )  # [1023] heap indices, level-major LSB-first-pos order
    feat2 = features.reshape(NUM_TREES, NUM_NODES)[:, colidx].astype(np.uint8)
    thr2 = thresholds.reshape(NUM_TREES, NUM_NODES)[:, colidx]
    # padded layout: [sm 0..126][pad 127][l7 128..255][l8 256..511][l9 512..1023]
    feat_lv = np.zeros((NUM_TREES, NCOL), np.uint8)
    thr_lv = np.full((NUM_TREES, NCOL), 1e30, np.float32)
    src_off = {d: (2 ** d - 1) for d in range(MAX_TREE_DEPTH)}
    for d in range(MAX_TREE_DEPTH):
        n = 2 ** d
        feat_lv[:, _OFF[d] : _OFF[d] + n] = feat2[:, src_off[d] : src_off[d] + n]
        thr_lv[:, _OFF[d] : _OFF[d] + n] = thr2[:, src_off[d] : src_off[d] + n]
    br10 = _bitrev_table(MAX_TREE_DEPTH)
    pv = br10[:512].astype(np.float16)  # 10-bit leaf offset (b9 adds +1)
    return feat_lv, thr_lv, pv


def make_in_maps(x, feat_lv, thr_lv, pv):
    in_maps = []
    tpc = NUM_TREES // N_CORES
    for c in range(N_CORES):
        xc = x[c * BC : (c + 1) * BC]  # [4096, 256]
        xt = np.ascontiguousarray(xc.T)  # [256, 4096]
        m = {
            "feat": np.ascontiguousarray(feat_lv[c * tpc : (c + 1) * tpc]),
            "thr": np.ascontiguousarray(thr_lv[c * tpc : (c + 1) * tpc]),
            "pv": pv,
            "xt": xt.reshape(2, 128, BC),
        }
        in_maps.append(m)
    return in_maps


def postprocess(results, values):
    vals3 = values.reshape(NUM_TREES, NUM_NODES, N_CLASSES)
    outs = []
    tix = np.arange(NUM_TREES)[None, :]
    shift = (2 * (np.arange(NUM_TREES) % 4)).astype(np.uint8)
    for c in range(N_CORES):
        lo = results[c]["olo"].astype(np.int32)  # [128, 32, 256]
        hp = results[c]["ohi"].astype(np.int32)  # [128, 32, 64]
        hi = (hp[:, :, np.arange(NUM_TREES) // 4] >> shift) & 3
        leaf = 1023 + lo + (hi << 8)  # heap node index 1023..2046
        leaf = leaf.transpose(1, 0, 2).reshape(BC, NUM_TREES)
        outs.append(vals3[tix, leaf])  # [BC, T, 8]
    return np.concatenate(outs, axis=0)


def _install_neff_cache():
    """Content-keyed disk cache for the walrus BIR->NEFF compile (the stock
    hook recompiles from scratch in every fresh process, ~40-130s)."""
    import hashlib
    import os
    import pathlib
    import shutil

    from concourse import bass2jax, bass_utils

    if getattr(bass2jax.compile_bir_kernel, "_neff_cache", False):
        return
    cache_dir = pathlib.Path.home() / ".cache" / "bass_neff_cache"
    try:
        cache_dir.mkdir(parents=True, exist_ok=True)
    except OSError:
        return
    orig = bass_utils.compile_bir_kernel

    def cached(bir_json, tmpdir, neff_name="file.neff"):
        h = hashlib.sha256(bir_json).hexdigest()
        cpath = cache_dir / f"{h}.neff"
        if cpath.exists():
            out = os.path.join(tmpdir, neff_name)
            shutil.copyfile(cpath, out)
            return out
        r = orig(bir_json, tmpdir, neff_name)
        try:
            shutil.copyfile(r, cpath.with_suffix(".tmp"))
            os.replace(cpath.with_suffix(".tmp"), cpath)
        except OSError:
            pass
        return r

    cached._neff_cache = True
    bass2jax.compile_bir_kernel = cached


def _make_runner(nc):
    """Like bass2jax.run_bass_via_pjrt, but the jitted executable is built
    once and reused across calls (no per-call retrace/recompile)."""
    import jax
    from jax.experimental.shard_map import shard_map
    from jax.sharding import Mesh, PartitionSpec
    from concourse import bass2jax

    bass2jax.install_neuronx_cc_hook()
    _install_neff_cache()
    assert nc.dbg_addr is None

    partition_name = (
        nc.partition_id_tensor.name if nc.partition_id_tensor else None
    )
    in_names = []
    out_names = []
    out_avals = []
    zero_shapes = []
    for alloc in nc.m.functions[0].allocations:
        if not isinstance(alloc, mybir.MemoryLocationSet):
            continue
        name = alloc.memorylocations[0].name
        if alloc.kind == "ExternalInput":
            if name != partition_name:
                in_names.append(name)
        elif alloc.kind == "ExternalOutput":
            out_names.append(name)
            shape = tuple(alloc.tensor_shape)
            dtype = mybir.dt.np(alloc.dtype)
            out_avals.append(jax.core.ShapedArray(shape, dtype))
            zero_shapes.append((shape, dtype))
    n_params = len(in_names)
    all_names = in_names + out_names
    if partition_name is not None:
        all_names = all_names + [partition_name]

    def _body(*args):
        operands = list(args)
        if partition_name is not None:
            operands.append(bass2jax.partition_id_tensor())
        outs = bass2jax._bass_exec_p.bind(
            *operands,
            out_avals=tuple(out_avals),
            in_names=tuple(all_names),
            out_names=tuple(out_names),
            lowering_input_output_aliases=(),
            sim_require_finite=True,
            sim_require_nnan=True,
            nc=nc,
        )
        return tuple(outs)

    devices = jax.devices()[:N_CORES]
    mesh = Mesh(np.asarray(devices), ("core",))
    n_outs = len(out_names)
    sharded = jax.jit(
        shard_map(
            _body, mesh=mesh,
            in_specs=(PartitionSpec("core"),) * (n_params + n_outs),
            out_specs=(PartitionSpec("core"),) * n_outs,
            check_rep=False,
        ),
        donate_argnums=tuple(range(n_params, n_params + n_outs)),
        keep_unused=True,
    )

    def run(in_maps):
        concat_in = [
            np.concatenate([np.asarray(m[nm]) for m in in_maps], axis=0)
            for nm in in_names
        ]
        concat_zeros = [
            np.zeros((N_CORES * s[0], *s[1:]), dt) for s, dt in zero_shapes
        ]
        out_arrs = sharded(*concat_in, *concat_zeros)
        return [
            {
                nm: np.asarray(out_arrs[i]).reshape(
                    N_CORES, *out_avals[i].shape
                )[c]
                for i, nm in enumerate(out_names)
            }
            for c in range(N_CORES)
        ]

    return run


def get_runner():
    if "runner" not in _PROGRAM_CACHE:
        _PROGRAM_CACHE["prog"] = build_program()
        _PROGRAM_CACHE["runner"] = _make_runner(_PROGRAM_CACHE["prog"])
    return _PROGRAM_CACHE["runner"]


def kernel(x, lefts, rights, features, thresholds, values, nodes_offset):
    x = np.asarray(x, dtype=np.float32)
    features = np.asarray(features, dtype=np.int32)
    thresholds = np.asarray(thresholds, dtype=np.float32)
    values = np.asarray(values, dtype=np.float32)

    feat_lv, thr_lv, pv = prep_tables(features, thresholds)
    in_maps = make_in_maps(x, feat_lv, thr_lv, pv)
    run = get_runner()

    results = None
    last_err = None
    for _attempt in range(3):
        try:
            results = run(in_maps)
            break
        except Exception as e:  # transient NRT device issues
            last_err = e
    if results is None:
        raise last_err
    return postprocess(results, values)
